# revision 32
# baseline (speedup 1.0000x reference)
"""Trainium2 Bass kernel for nn_MergedLinearFormer.

Computes out = softmax((x@QK)@x^T / sqrt(D)) @ x @ VO for x:[B,T,D].

Sharding: 8 cores; core c handles batch b=c//2, query half h=c%2 (2048
queries each). The host uploads one bf16 copy of x[b] per core, with the
rows rotated so this core's queries are rows 0..TQ-1 (key order is a
permutation, which softmax+AV are invariant to), plus the full 8 MB
transpose x^T (pre-transposed on the host; the on-chip xbar runs at only
~80 GB/s, so host transposition keeps the whole input stream on the fast
plain-DMA path). x^T stays SBUF-resident so the S-phase reads SBUF
directly and needs no per-chunk DMA.

Inside a core, everything is computed with the score matrix TRANSPOSED
(keys on PSUM partitions, queries on the free axis) so no on-chip
transposes are needed anywhere:

  phase 1:  xQK^T[e, q]   = QK^T @ xq^T          (lhsT=QK,  rhs=xT cols)
  S-phase:  S^T[u, q]     = x @ xQK^T             (lhsT=xT,  rhs=xQK^T)
            P^T[u, q]     = exp(S^T / sqrt(D))    (no max subtraction:
                             scores are ~N(0,1), exp can't overflow)
            colsum[q]    += P^T                   (DVE partial sums)
            den_j[q, 1]   = colsum_j^T @ ones     (tiny matmuls put the
                             denominators on partitions, no scatter DMA)
  AV-phase: av^T[d, q]    = x^T @ P^T             (lhsT=x,   rhs=P^T)
  OUT:      out[q, e]     = (av^T)^T @ VO         (lhsT=av^T, rhs=VO)
            out[q, e]    *= 1/den[q]

Startup is tuned around two HW facts seen in the trace: (a) the PE
clock ramps (0.65->1.2->2.4 GHz over the first ~3us of a busy streak),
so a handful of warm-up matmuls on memset scratch run while the first
input DMAs are still in flight; (b) phase 1 emitted et-major can only
finish its first PSUM tile after ALL of QK (2 MB) has landed, which
stalled the PE ~3.5us. Phase 1 for query block 0 is therefore emitted
kt-MAJOR across all 8 PSUM banks: round kt needs only one 384 KB
(QK-tile, xT-piece) pair, which DMA delivers faster than the PE
consumes it. The PSUM->SBUF copies are interleaved into the last round
(alternating ACT/DVE) so the S-phase can chase them tile by tile.

Loads ride the two HWDGE queues in strict order of first use (QK on
the ACT engine's queue; everything else on sync's — splitting the xT
stream across both queues was tried and starves the urgent pieces).
The drain end is handled with care: a HWDGE queue that has gone idle
takes ~1.5us to dispatch a fresh DMA (wake + descriptor generation),
so the last 512-col OUT chain is split into two 256-col chains whose
stores land on alternating queues — the first store's dispatch hides
under the second chain's matmuls.

All matmul operands are bf16 (PE streams 1 column/cycle regardless of
dtype, so bf16 halves DMA/SBUF at no PE cost); accumulation is fp32 in
PSUM; output bf16 (converted to fp32 on the host).
"""

import numpy as np
import ml_dtypes

import concourse.mybir as mybir
import concourse.tile as tile
from concourse import bacc

P = 128
B, T, D = 4, 4096, 1024
TQ = T // 2          # queries per core
CH = 512             # query-chunk width
ET = D // P          # 8 tiles along the model dim
UT = T // P          # 32 tiles along the key dim
UG = T // 512        # 8 key groups of 512
CHUNKS = TQ // CH    # 4
JT = CH // P         # 4 query tiles per chunk
SCALE = 1.0 / np.sqrt(D)

BF16 = mybir.dt.bfloat16
F32 = mybir.dt.float32
NPBF16 = ml_dtypes.bfloat16


def _build():
    nc = bacc.Bacc()
    x_ = nc.dram_tensor("x", [T, D], BF16, kind="ExternalInput")
    xTd = nc.dram_tensor("xT", [D, T], BF16, kind="ExternalInput")
    QK = nc.dram_tensor("QK", [D, D], BF16, kind="ExternalInput")
    VO = nc.dram_tensor("VO", [D, D], BF16, kind="ExternalInput")
    out = nc.dram_tensor("out", [TQ, D], BF16, kind="ExternalOutput")

    x_r = x_.rearrange("(uo p) d -> p uo d", p=P)       # [128, 32, 1024]
    xT_r = xTd.rearrange("(eo p) u -> p eo u", p=P)     # [128, 8, 4096]
    QK_r = QK.rearrange("(ko p) e -> p ko e", p=P)      # [128, 8, 1024]
    VO_r = VO.rearrange("(ko p) e -> p ko e", p=P)      # [128, 8, 1024]

    with tile.TileContext(nc) as tc:
        with (
            tc.tile_pool(name="resident", bufs=1) as resident,
            tc.tile_pool(name="consts", bufs=1) as consts,
            tc.tile_pool(name="ptpool", bufs=UT + 2) as ptpool,
            tc.tile_pool(name="xpan_pool", bufs=4) as xpan_pool,
            tc.tile_pool(name="avpool", bufs=2) as avpool,
            tc.tile_pool(name="outpool", bufs=2) as outpool,
            tc.tile_pool(name="small", bufs=2) as small,
            tc.tile_pool(name="ps_acc", bufs=3, space="PSUM") as ps_acc_pool,
            tc.tile_pool(name="ps_sums", bufs=1, space="PSUM") as ps_sums_pool,
            tc.tile_pool(name="ps_av", bufs=2, space="PSUM") as ps_av_pool,
            tc.tile_pool(name="ps_o", bufs=2, space="PSUM") as ps_o_pool,
        ):
            # SBUF-resident transposed x and xQK^T.
            xT = resident.tile([P, ET, T], BF16)      # 8 MB
            xqkt = resident.tile([P, ET, TQ], BF16)   # 4 MB
            qk_sb = resident.tile([P, ET, D], BF16)   # 2 MB
            scratch = consts.tile([P, 512], BF16)     # PE warm-up fodder
            ones_sb = consts.tile([P, 1], F32)
            vo_sb = consts.tile([P, ET, D], BF16)

            nc.gpsimd.memset(scratch, 0.0)
            nc.vector.memset(ones_sb, 1.0)

            # ---- loads, in order of first use ----
            # Critical pairs first: phase-1 round kt consumes exactly
            # (qk tile kt, xT[:, kt, 0:512]).  QK goes via the scalar
            # engine's queue, the whole xT stream via sync's, in strict
            # order of first use: query cols 512:2048 (S ut4-15 consume
            # them from ~30us), then the key half (S ut16+ from ~50us),
            # then VO (OUT phase, >150us).  Splitting the xT stream
            # across both queues starves the urgent pieces — tried it.
            for kt in range(ET):
                nc.scalar.dma_start(qk_sb[:, kt, :], QK_r[:, kt, :])
                nc.sync.dma_start(xT[:, kt, 0:512], xT_r[:, kt, 0:512])
            for ug in range(1, UG):
                for kt in range(ET):
                    nc.sync.dma_start(
                        xT[:, kt, ug * 512 : (ug + 1) * 512],
                        xT_r[:, kt, ug * 512 : (ug + 1) * 512],
                    )
            nc.sync.dma_start(vo_sb, VO_r)

            # ---- PE clock warm-up ----
            # The tensor engine ramps 0.65 -> 1.2 -> 2.4 GHz over the
            # first ~3us of a busy streak.  Burn the DMA-bound startup
            # window on scratch matmuls so the real phase-1 rounds run
            # at full clock from their first instruction.
            warm_ps = ps_o_pool.tile([P, 512], F32, name="o_ps")
            for _ in range(8):
                nc.tensor.matmul(
                    warm_ps, scratch[:, 0:P], scratch, start=True, stop=True
                )

            # ---- phase 1 for query block 0: kt-major over 8 banks ----
            # Each round kt touches one (qk[kt], xT[:,kt,0:512]) pair,
            # so the PE never waits for the whole 2 MB of QK.  Copies
            # to xqkt ride the last round, alternating ACT/DVE, in et
            # order -- exactly the order the S-phase consumes them.
            ps8 = (
                [ps_acc_pool.tile([P, 512], F32, name="acc_ps") for _ in range(3)]
                + [ps_sums_pool.tile([P, 512], F32, name="den_ps")]
                + [ps_av_pool.tile([P, 512], F32, name="av_ps") for _ in range(2)]
                + [ps_o_pool.tile([P, 512], F32, name="o_ps") for _ in range(2)]
            )
            for kt in range(ET):
                for et in range(ET):
                    nc.tensor.matmul(
                        ps8[et],
                        qk_sb[:, kt, et * P : (et + 1) * P],
                        xT[:, kt, 0:512],
                        start=(kt == 0),
                        stop=(kt == ET - 1),
                    )
                    if kt == ET - 1:
                        if et % 2 == 0:
                            nc.scalar.copy(xqkt[:, et, 0:512], ps8[et])
                        else:
                            nc.vector.tensor_copy(xqkt[:, et, 0:512], ps8[et])

            # ---- phase 1 for query blocks 1-3 (steady state) ----
            def ph1_nq(nq):
                for et in range(ET):
                    ps = ps_acc_pool.tile([P, 512], F32, name="acc_ps")
                    for kt in range(ET):
                        nc.tensor.matmul(
                            ps,
                            qk_sb[:, kt, et * P : (et + 1) * P],
                            xT[:, kt, nq * 512 : (nq + 1) * 512],
                            start=(kt == 0),
                            stop=(kt == ET - 1),
                        )
                    nc.scalar.copy(xqkt[:, et, nq * 512 : (nq + 1) * 512], ps)

            # ---- S-phase tile: S^T, exp, denominator accumulation ----
            def s_tile(ut, q0, colsum, pts):
                s_ps = ps_acc_pool.tile([P, CH], F32, name="acc_ps")
                for kt in range(ET):
                    nc.tensor.matmul(
                        s_ps,
                        xT[:, kt, ut * P : (ut + 1) * P],
                        xqkt[:, kt, q0 : q0 + CH],
                        start=(kt == 0),
                        stop=(kt == ET - 1),
                    )
                pt = ptpool.tile([P, CH], BF16, name="pt")
                nc.scalar.activation(
                    pt, s_ps, mybir.ActivationFunctionType.Exp, scale=SCALE
                )
                if ut == 0:
                    nc.vector.tensor_copy(colsum, pt)
                else:
                    nc.vector.tensor_add(colsum, colsum, pt)
                pts.append(pt)

            for c in range(CHUNKS):
                q0 = c * CH
                colsum = small.tile([P, CH], F32, name="colsum")
                pts = []
                for ut in range(UT):
                    s_tile(ut, q0, colsum, pts)
                if c == 0:
                    # The remaining phase-1 blocks run here, after
                    # chunk 0's S-phase: by now the whole input stream
                    # is resident, and chunk 1's S-phase needs xqkt
                    # nq1 only after chunk 0's AV+OUT (~55us away).
                    for nq in range(1, CHUNKS):
                        ph1_nq(nq)

                # ---- AV-phase: av^T[d, q] ----
                # x panels stream in 512 KB halves so a late DMA only
                # stalls 16 of the 32 accumulation matmuls.
                av_sb = avpool.tile([P, ET, CH], BF16, name="av_sb")
                UH = UT // 2
                for dt in range(ET):
                    av_ps = ps_av_pool.tile([P, CH], F32, name="av_ps")
                    for uh in range(2):
                        x_pan = xpan_pool.tile([P, UH, P], BF16, name="x_pan")
                        nc.sync.dma_start(
                            x_pan,
                            x_r[:, uh * UH : (uh + 1) * UH, dt * P : (dt + 1) * P],
                        )
                        for ui in range(UH):
                            ut = uh * UH + ui
                            nc.tensor.matmul(
                                av_ps,
                                x_pan[:, ui, :],
                                pts[ut],
                                start=(ut == 0),
                                stop=(ut == UT - 1),
                            )
                    nc.vector.tensor_copy(av_sb[:, dt, :], av_ps)

                # ---- denominators -> per-partition reciprocals ----
                # Emitted AFTER the AV matmuls: these tiny matmuls wait on
                # the ACT exp of the last S tile (via colsum), and the PE
                # queue is strictly in-order — placed between S and AV
                # they stall the AV start ~1 us per chunk. Here they slot
                # between AV and OUT, and the reciprocal easily beats
                # OUT's first normalization. Four matmuls write disjoint
                # columns of one PSUM bank (start only on the first:
                # later writes land on never-written elements, so they
                # overwrite, not add).
                den_ps = ps_sums_pool.tile([P, JT], F32, name="den_ps")
                for j in range(JT):
                    nc.tensor.matmul(
                        den_ps[:, j : j + 1],
                        colsum[:, j * P : (j + 1) * P],
                        ones_sb,
                        start=(j == 0),
                        stop=(j == JT - 1),
                    )
                r_sb = small.tile([P, JT], F32, name="r_sb")
                nc.vector.reciprocal(r_sb, den_ps)

                # ---- OUT: (av^T)^T @ VO, normalized ----
                store_flip = 0
                for j in range(JT):
                    out_sb = outpool.tile([P, D], BF16, name="out_sb")
                    rows = slice(q0 + j * P, q0 + (j + 1) * P)
                    for eh in range(2):
                        last = c == CHUNKS - 1 and j == JT - 1 and eh == 1
                        if last:
                            # Final piece: run it as two 256-col chains.
                            # The first half's normalize+store (and the
                            # ~1.5us queue-wake latency of its dispatch)
                            # overlap the second half's matmuls, so only
                            # a 256-col store remains after the last
                            # matmul — on the scalar queue, which is
                            # still hot from the earlier stores.
                            for qh in range(2):
                                o_ps = ps_o_pool.tile([P, 256], F32, name="o_ps")
                                for dt in range(ET):
                                    nc.tensor.matmul(
                                        o_ps,
                                        av_sb[:, dt, j * P : (j + 1) * P],
                                        vo_sb[
                                            :,
                                            dt,
                                            512 + qh * 256 : 512 + (qh + 1) * 256,
                                        ],
                                        start=(dt == 0),
                                        stop=(dt == ET - 1),
                                    )
                                sl = slice(512 + qh * 256, 512 + (qh + 1) * 256)
                                nc.vector.tensor_scalar_mul(
                                    out_sb[:, sl], o_ps, r_sb[:, j : j + 1]
                                )
                                if qh == 0:
                                    nc.scalar.dma_start(
                                        out[rows, sl], out_sb[:, sl]
                                    )
                                else:
                                    # Very last store: split by rows so
                                    # each queue generates only 64
                                    # descriptors (~11ns/row) in
                                    # parallel.
                                    r0 = q0 + j * P
                                    nc.sync.dma_start(
                                        out[r0 : r0 + 64, sl],
                                        out_sb[0:64, sl],
                                    )
                                    nc.scalar.dma_start(
                                        out[r0 + 64 : r0 + 128, sl],
                                        out_sb[64:128, sl],
                                    )
                                store_flip += 1
                            continue
                        o_ps = ps_o_pool.tile([P, 512], F32, name="o_ps")
                        for dt in range(ET):
                            nc.tensor.matmul(
                                o_ps,
                                av_sb[:, dt, j * P : (j + 1) * P],
                                vo_sb[:, dt, eh * 512 : (eh + 1) * 512],
                                start=(dt == 0),
                                stop=(dt == ET - 1),
                            )
                        nc.vector.tensor_scalar_mul(
                            out_sb[:, eh * 512 : (eh + 1) * 512],
                            o_ps,
                            r_sb[:, j : j + 1],
                        )
                        if c == CHUNKS - 1:
                            # Last chunk: store each half as soon as its
                            # normalization lands, alternating queues so
                            # neither backs up at the drain.
                            eng = nc.sync if store_flip % 2 == 0 else nc.scalar
                            store_flip += 1
                            eng.dma_start(
                                out[rows, eh * 512 : (eh + 1) * 512],
                                out_sb[:, eh * 512 : (eh + 1) * 512],
                            )
                    if c != CHUNKS - 1:
                        # Scalar engine's queue: keeps the sync queue
                        # exclusively on x-panel prefetch, so stores
                        # never delay the next chunk's AV data.
                        nc.scalar.dma_start(out[rows, :], out_sb)

    nc.compile()
    return nc


_NC = None


def _get_nc():
    global _NC
    if _NC is None:
        _NC = _build()
    return _NC


_RUNNER = None


def _get_runner():
    """Cached jitted 8-core SPMD executor (avoids re-tracing per call)."""
    global _RUNNER
    if _RUNNER is not None:
        return _RUNNER

    import jax
    import jax.numpy as jnp
    from jax.sharding import Mesh, NamedSharding, PartitionSpec
    from jax.experimental.shard_map import shard_map
    from concourse import bass2jax

    nc = _get_nc()
    bass2jax.install_neuronx_cc_hook()
    partition_name = nc.partition_id_tensor.name if nc.partition_id_tensor else None
    in_names, out_names, out_avals = [], [], []
    for alloc in nc.m.functions[0].allocations:
        if not isinstance(alloc, mybir.MemoryLocationSet):
            continue
        name = alloc.memorylocations[0].name
        if alloc.kind == "ExternalInput":
            if name != partition_name:
                in_names.append(name)
        elif alloc.kind == "ExternalOutput":
            shape = tuple(alloc.tensor_shape)
            dtype = mybir.dt.np(alloc.dtype)
            out_names.append(name)
            out_avals.append(jax.core.ShapedArray(shape, dtype))
    n_params = len(in_names)
    n_outs = len(out_avals)
    in_names_all = in_names + out_names
    if partition_name is not None:
        in_names_all = in_names_all + [partition_name]

    def _body(*args):
        operands = list(args)
        if partition_name is not None:
            operands.append(bass2jax.partition_id_tensor())
        return tuple(
            bass2jax._bass_exec_p.bind(
                *operands,
                out_avals=tuple(out_avals),
                in_names=tuple(in_names_all),
                out_names=tuple(out_names),
                lowering_input_output_aliases=(),
                sim_require_finite=True,
                sim_require_nnan=True,
                nc=nc,
            )
        )

    devices = jax.devices()[:8]
    mesh = Mesh(np.asarray(devices), ("core",))
    in_specs = (PartitionSpec("core"),) * (n_params + n_outs)
    out_specs = (PartitionSpec("core"),) * len(out_names)
    donate = tuple(range(n_params, n_params + n_outs))
    sharded = jax.jit(
        shard_map(
            _body, mesh=mesh, in_specs=in_specs, out_specs=out_specs, check_rep=False
        ),
        donate_argnums=donate,
        keep_unused=True,
    )
    shard = NamedSharding(mesh, PartitionSpec("core"))

    # Donated output buffers are created on-device (a trivial jitted zeros
    # program) instead of being uploaded from the host every call.
    zeros_fns = [
        jax.jit(
            lambda aval=aval: jnp.zeros((8 * aval.shape[0], *aval.shape[1:]), aval.dtype),
            out_shardings=shard,
        )
        for aval in out_avals
    ]

    def run(in_maps):
        per_core = [[np.asarray(m[nm]) for nm in in_names] for m in in_maps]
        concat_in = [
            np.concatenate([per_core[c][i] for c in range(8)], axis=0)
            for i in range(n_params)
        ]
        dev_zeros = [fn() for fn in zeros_fns]
        out_arrs = sharded(*concat_in, *dev_zeros)
        return [
            {
                name: np.asarray(out_arrs[i]).reshape(8, *out_avals[i].shape)[c]
                for i, name in enumerate(out_names)
            }
            for c in range(8)
        ]

    _RUNNER = run
    return _RUNNER


def _make_in_maps(inputs):
    x = np.asarray(inputs["x"], dtype=np.float32)
    QK16 = np.asarray(inputs["QK"], dtype=np.float32).astype(NPBF16)
    VO16 = np.asarray(inputs["VO"], dtype=np.float32).astype(NPBF16)
    xb16 = [np.ascontiguousarray(x[b].astype(NPBF16)) for b in range(B)]
    in_maps = []
    for c in range(8):
        b, h = divmod(c, 2)
        if h == 0:
            xc = xb16[b]
        else:
            # Rotate rows so this core's queries are rows 0..TQ-1; the key
            # order is permuted identically in S and AV, which softmax and
            # the attention average are invariant to.
            xc = np.concatenate([xb16[b][TQ:], xb16[b][:TQ]], axis=0)
        xTc = np.ascontiguousarray(xc.T)
        in_maps.append({"x": xc, "xT": xTc, "QK": QK16, "VO": VO16})
    return in_maps


def kernel(x, QK, VO):
    in_maps = _make_in_maps({"x": x, "QK": QK, "VO": VO})
    results = _get_runner()(in_maps)
    out = np.empty((B, T, D), dtype=np.float32)
    for c in range(8):
        b, h = divmod(c, 2)
        out[b, h * TQ : (h + 1) * TQ, :] = results[c]["out"].astype(np.float32)
    return out


# revision 33
# speedup vs baseline: 1.1079x; 1.1079x over previous
"""Trainium2 Bass kernel for nn_MergedLinearFormer.

Computes out = softmax((x@QK)@x^T / sqrt(D)) @ x @ VO for x:[B,T,D].

Sharding: 8 cores; core c handles batch b=c//2, query half h=c%2 (2048
queries each). The host uploads one bf16 copy of x[b] per core, with the
rows rotated so this core's queries are rows 0..TQ-1 (key order is a
permutation, which softmax+AV are invariant to), plus the full 8 MB
transpose x^T (pre-transposed on the host; the on-chip xbar runs at only
~80 GB/s, so host transposition keeps the whole input stream on the fast
plain-DMA path). x^T stays SBUF-resident so the S-phase reads SBUF
directly and needs no per-chunk DMA.

Inside a core, everything is computed with the score matrix TRANSPOSED
(keys on PSUM partitions, queries on the free axis) so no on-chip
transposes are needed anywhere:

  phase 1:  xQK^T[e, q]   = QK^T @ xq^T          (lhsT=QK,  rhs=xT cols)
  S-phase:  S^T[u, q]     = x @ xQK^T             (lhsT=xT,  rhs=xQK^T)
            P^T[u, q]     = exp(S^T / sqrt(D))    (no max subtraction:
                             scores are ~N(0,1), exp can't overflow)
            colsum[q]    += P^T                   (DVE partial sums)
            den_j[q, 1]   = colsum_j^T @ ones     (tiny matmuls put the
                             denominators on partitions, no scatter DMA)
  AV-phase: av^T[d, q]    = x^T @ P^T             (lhsT=x,   rhs=P^T)
  OUT:      out[q, e]     = (av^T)^T @ VO         (lhsT=av^T, rhs=VO)
            out[q, e]    *= 1/den[q]

Startup is tuned around two HW facts seen in the trace: (a) the PE
clock ramps (0.65->1.2->2.4 GHz over the first ~3us of a busy streak),
so a handful of warm-up matmuls on memset scratch run while the first
input DMAs are still in flight; (b) phase 1 emitted et-major can only
finish its first PSUM tile after ALL of QK (2 MB) has landed, which
stalled the PE ~3.5us. Phase 1 for query block 0 is therefore emitted
kt-MAJOR across all 8 PSUM banks: round kt needs only one 384 KB
(QK-tile, xT-piece) pair, which DMA delivers faster than the PE
consumes it. The PSUM->SBUF copies are interleaved into the last round
(alternating ACT/DVE) so the S-phase can chase them tile by tile.

Loads ride the two HWDGE queues in strict order of first use (QK on
the ACT engine's queue; everything else on sync's — splitting the xT
stream across both queues was tried and starves the urgent pieces).
The drain end is handled with care: a HWDGE queue that has gone idle
takes ~1.5us to dispatch a fresh DMA (wake + descriptor generation),
so the last 512-col OUT chain is split into two 256-col chains whose
stores land on alternating queues — the first store's dispatch hides
under the second chain's matmuls.

All matmul operands are bf16 (PE streams 1 column/cycle regardless of
dtype, so bf16 halves DMA/SBUF at no PE cost); accumulation is fp32 in
PSUM; output bf16 (converted to fp32 on the host).
"""

import numpy as np
import ml_dtypes

import concourse.mybir as mybir
import concourse.tile as tile
from concourse import bacc

P = 128
B, T, D = 4, 4096, 1024
TQ = T // 2          # queries per core
CH = 512             # query-chunk width
ET = D // P          # 8 tiles along the model dim
UT = T // P          # 32 tiles along the key dim
UG = T // 512        # 8 key groups of 512
CHUNKS = TQ // CH    # 4
JT = CH // P         # 4 query tiles per chunk
SCALE = 1.0 / np.sqrt(D)

BF16 = mybir.dt.bfloat16
F32 = mybir.dt.float32
NPBF16 = ml_dtypes.bfloat16


def _build():
    nc = bacc.Bacc()
    x_ = nc.dram_tensor("x", [T, D], BF16, kind="ExternalInput")
    xTd = nc.dram_tensor("xT", [D, T], BF16, kind="ExternalInput")
    QK = nc.dram_tensor("QK", [D, D], BF16, kind="ExternalInput")
    VO = nc.dram_tensor("VO", [D, D], BF16, kind="ExternalInput")
    out = nc.dram_tensor("out", [TQ, D], BF16, kind="ExternalOutput")

    x_r = x_.rearrange("(uo p) d -> p uo d", p=P)       # [128, 32, 1024]
    xT_r = xTd.rearrange("(eo p) u -> p eo u", p=P)     # [128, 8, 4096]
    QK_r = QK.rearrange("(ko p) e -> p ko e", p=P)      # [128, 8, 1024]
    VO_r = VO.rearrange("(ko p) e -> p ko e", p=P)      # [128, 8, 1024]

    with tile.TileContext(nc) as tc:
        with (
            tc.tile_pool(name="resident", bufs=1) as resident,
            tc.tile_pool(name="consts", bufs=1) as consts,
            tc.tile_pool(name="ptpool", bufs=UT + 2) as ptpool,
            tc.tile_pool(name="xpan_pool", bufs=4) as xpan_pool,
            tc.tile_pool(name="avpool", bufs=2) as avpool,
            tc.tile_pool(name="outpool", bufs=2) as outpool,
            tc.tile_pool(name="small", bufs=2) as small,
            tc.tile_pool(name="ps_acc", bufs=3, space="PSUM") as ps_acc_pool,
            tc.tile_pool(name="ps_sums", bufs=1, space="PSUM") as ps_sums_pool,
            tc.tile_pool(name="ps_av", bufs=2, space="PSUM") as ps_av_pool,
            tc.tile_pool(name="ps_o", bufs=2, space="PSUM") as ps_o_pool,
        ):
            # SBUF-resident transposed x and xQK^T.
            xT = resident.tile([P, ET, T], BF16)      # 8 MB
            xqkt = resident.tile([P, ET, TQ], BF16)   # 4 MB
            qk_sb = resident.tile([P, ET, D], BF16)   # 2 MB
            scratch = consts.tile([P, 512], BF16)     # PE warm-up fodder
            ones_sb = consts.tile([P, 1], F32)
            vo_sb = consts.tile([P, ET, D], BF16)

            nc.gpsimd.memset(scratch, 0.0)
            nc.vector.memset(ones_sb, 1.0)

            # ---- loads, in order of first use ----
            # Critical pairs first: phase-1 round kt consumes exactly
            # (qk tile kt, xT[:, kt, 0:512]).  QK goes via the SYNC
            # engine's queue — the scalar/ACT engine spends its first
            # 1.28us on ACT_TABLE_LOAD, which would delay the very
            # first dispatch — and the xT stream via scalar's, in
            # strict order of first use: query cols 512:2048 (S ut4-15
            # consume them from ~30us), then the key half (S ut16+
            # from ~50us).  VO (OUT phase, >150us) follows QK on sync.
            # Splitting ONE stream across both queues starves the
            # urgent pieces — tried it.
            for kt in range(ET):
                nc.sync.dma_start(qk_sb[:, kt, :], QK_r[:, kt, :])
                nc.scalar.dma_start(xT[:, kt, 0:512], xT_r[:, kt, 0:512])
            for ug in range(1, UG):
                for kt in range(ET):
                    nc.scalar.dma_start(
                        xT[:, kt, ug * 512 : (ug + 1) * 512],
                        xT_r[:, kt, ug * 512 : (ug + 1) * 512],
                    )
            nc.sync.dma_start(vo_sb, VO_r)

            # ---- PE clock warm-up ----
            # The tensor engine ramps 0.65 -> 1.2 -> 2.4 GHz over the
            # first ~3us of a busy streak.  Burn the DMA-bound startup
            # window on scratch matmuls so the real phase-1 rounds run
            # at full clock from their first instruction.
            warm_ps = ps_o_pool.tile([P, 512], F32, name="o_ps")
            for _ in range(8):
                nc.tensor.matmul(
                    warm_ps, scratch[:, 0:P], scratch, start=True, stop=True
                )

            # ---- phase 1 for query block 0: kt-major over 8 banks ----
            # Each round kt touches one (qk[kt], xT[:,kt,0:512]) pair,
            # so the PE never waits for the whole 2 MB of QK.  Copies
            # to xqkt ride the last round, alternating ACT/DVE, in et
            # order -- exactly the order the S-phase consumes them.
            ps8 = (
                [ps_acc_pool.tile([P, 512], F32, name="acc_ps") for _ in range(3)]
                + [ps_sums_pool.tile([P, 512], F32, name="den_ps")]
                + [ps_av_pool.tile([P, 512], F32, name="av_ps") for _ in range(2)]
                + [ps_o_pool.tile([P, 512], F32, name="o_ps") for _ in range(2)]
            )
            for kt in range(ET):
                for et in range(ET):
                    nc.tensor.matmul(
                        ps8[et],
                        qk_sb[:, kt, et * P : (et + 1) * P],
                        xT[:, kt, 0:512],
                        start=(kt == 0),
                        stop=(kt == ET - 1),
                    )
                    if kt == ET - 1:
                        if et % 2 == 0:
                            nc.scalar.copy(xqkt[:, et, 0:512], ps8[et])
                        else:
                            nc.vector.tensor_copy(xqkt[:, et, 0:512], ps8[et])

            # ---- phase 1 for query blocks 1-3 (steady state) ----
            def ph1_nq(nq):
                for et in range(ET):
                    ps = ps_acc_pool.tile([P, 512], F32, name="acc_ps")
                    for kt in range(ET):
                        nc.tensor.matmul(
                            ps,
                            qk_sb[:, kt, et * P : (et + 1) * P],
                            xT[:, kt, nq * 512 : (nq + 1) * 512],
                            start=(kt == 0),
                            stop=(kt == ET - 1),
                        )
                    nc.scalar.copy(xqkt[:, et, nq * 512 : (nq + 1) * 512], ps)

            # ---- S-phase tile: S^T, exp, denominator accumulation ----
            def s_tile(ut, q0, colsum, pts):
                s_ps = ps_acc_pool.tile([P, CH], F32, name="acc_ps")
                for kt in range(ET):
                    nc.tensor.matmul(
                        s_ps,
                        xT[:, kt, ut * P : (ut + 1) * P],
                        xqkt[:, kt, q0 : q0 + CH],
                        start=(kt == 0),
                        stop=(kt == ET - 1),
                    )
                pt = ptpool.tile([P, CH], BF16, name="pt")
                nc.scalar.activation(
                    pt, s_ps, mybir.ActivationFunctionType.Exp, scale=SCALE
                )
                if ut == 0:
                    nc.vector.tensor_copy(colsum, pt)
                else:
                    nc.vector.tensor_add(colsum, colsum, pt)
                pts.append(pt)

            for c in range(CHUNKS):
                q0 = c * CH
                colsum = small.tile([P, CH], F32, name="colsum")
                pts = []
                for ut in range(UT):
                    s_tile(ut, q0, colsum, pts)
                if c == 0:
                    # The remaining phase-1 blocks run here, after
                    # chunk 0's S-phase: by now the whole input stream
                    # is resident, and chunk 1's S-phase needs xqkt
                    # nq1 only after chunk 0's AV+OUT (~55us away).
                    for nq in range(1, CHUNKS):
                        ph1_nq(nq)

                # ---- AV-phase: av^T[d, q] ----
                # x panels stream in 512 KB halves so a late DMA only
                # stalls 16 of the 32 accumulation matmuls.
                av_sb = avpool.tile([P, ET, CH], BF16, name="av_sb")
                UH = UT // 2
                for dt in range(ET):
                    av_ps = ps_av_pool.tile([P, CH], F32, name="av_ps")
                    for uh in range(2):
                        x_pan = xpan_pool.tile([P, UH, P], BF16, name="x_pan")
                        nc.sync.dma_start(
                            x_pan,
                            x_r[:, uh * UH : (uh + 1) * UH, dt * P : (dt + 1) * P],
                        )
                        for ui in range(UH):
                            ut = uh * UH + ui
                            nc.tensor.matmul(
                                av_ps,
                                x_pan[:, ui, :],
                                pts[ut],
                                start=(ut == 0),
                                stop=(ut == UT - 1),
                            )
                    nc.vector.tensor_copy(av_sb[:, dt, :], av_ps)

                # ---- denominators -> per-partition reciprocals ----
                # Emitted AFTER the AV matmuls: these tiny matmuls wait on
                # the ACT exp of the last S tile (via colsum), and the PE
                # queue is strictly in-order — placed between S and AV
                # they stall the AV start ~1 us per chunk. Here they slot
                # between AV and OUT, and the reciprocal easily beats
                # OUT's first normalization. Four matmuls write disjoint
                # columns of one PSUM bank (start only on the first:
                # later writes land on never-written elements, so they
                # overwrite, not add).
                den_ps = ps_sums_pool.tile([P, JT], F32, name="den_ps")
                for j in range(JT):
                    nc.tensor.matmul(
                        den_ps[:, j : j + 1],
                        colsum[:, j * P : (j + 1) * P],
                        ones_sb,
                        start=(j == 0),
                        stop=(j == JT - 1),
                    )
                r_sb = small.tile([P, JT], F32, name="r_sb")
                nc.vector.reciprocal(r_sb, den_ps)

                # ---- OUT: (av^T)^T @ VO, normalized ----
                store_flip = 0
                for j in range(JT):
                    out_sb = outpool.tile([P, D], BF16, name="out_sb")
                    rows = slice(q0 + j * P, q0 + (j + 1) * P)
                    for eh in range(2):
                        last = c == CHUNKS - 1 and j == JT - 1 and eh == 1
                        if last:
                            # Final piece: run it as two 256-col chains.
                            # The first half's normalize+store (and the
                            # ~1.5us queue-wake latency of its dispatch)
                            # overlap the second half's matmuls, so only
                            # a 256-col store remains after the last
                            # matmul — on the scalar queue, which is
                            # still hot from the earlier stores.
                            for qh in range(2):
                                o_ps = ps_o_pool.tile([P, 256], F32, name="o_ps")
                                for dt in range(ET):
                                    nc.tensor.matmul(
                                        o_ps,
                                        av_sb[:, dt, j * P : (j + 1) * P],
                                        vo_sb[
                                            :,
                                            dt,
                                            512 + qh * 256 : 512 + (qh + 1) * 256,
                                        ],
                                        start=(dt == 0),
                                        stop=(dt == ET - 1),
                                    )
                                sl = slice(512 + qh * 256, 512 + (qh + 1) * 256)
                                nc.vector.tensor_scalar_mul(
                                    out_sb[:, sl], o_ps, r_sb[:, j : j + 1]
                                )
                                if qh == 0:
                                    nc.scalar.dma_start(
                                        out[rows, sl], out_sb[:, sl]
                                    )
                                else:
                                    # Very last store: split by rows so
                                    # each queue generates only 64
                                    # descriptors (~11ns/row) in
                                    # parallel.
                                    r0 = q0 + j * P
                                    nc.sync.dma_start(
                                        out[r0 : r0 + 64, sl],
                                        out_sb[0:64, sl],
                                    )
                                    nc.scalar.dma_start(
                                        out[r0 + 64 : r0 + 128, sl],
                                        out_sb[64:128, sl],
                                    )
                                store_flip += 1
                            continue
                        o_ps = ps_o_pool.tile([P, 512], F32, name="o_ps")
                        for dt in range(ET):
                            nc.tensor.matmul(
                                o_ps,
                                av_sb[:, dt, j * P : (j + 1) * P],
                                vo_sb[:, dt, eh * 512 : (eh + 1) * 512],
                                start=(dt == 0),
                                stop=(dt == ET - 1),
                            )
                        nc.vector.tensor_scalar_mul(
                            out_sb[:, eh * 512 : (eh + 1) * 512],
                            o_ps,
                            r_sb[:, j : j + 1],
                        )
                        if c == CHUNKS - 1:
                            # Last chunk: store each half as soon as its
                            # normalization lands, alternating queues so
                            # neither backs up at the drain.
                            eng = nc.sync if store_flip % 2 == 0 else nc.scalar
                            store_flip += 1
                            eng.dma_start(
                                out[rows, eh * 512 : (eh + 1) * 512],
                                out_sb[:, eh * 512 : (eh + 1) * 512],
                            )
                    if c != CHUNKS - 1:
                        # Scalar engine's queue: keeps the sync queue
                        # exclusively on x-panel prefetch, so stores
                        # never delay the next chunk's AV data.
                        nc.scalar.dma_start(out[rows, :], out_sb)

    nc.compile()
    return nc


_NC = None


def _get_nc():
    global _NC
    if _NC is None:
        _NC = _build()
    return _NC


_RUNNER = None


def _get_runner():
    """Cached jitted 8-core SPMD executor (avoids re-tracing per call)."""
    global _RUNNER
    if _RUNNER is not None:
        return _RUNNER

    import jax
    import jax.numpy as jnp
    from jax.sharding import Mesh, NamedSharding, PartitionSpec
    from jax.experimental.shard_map import shard_map
    from concourse import bass2jax

    nc = _get_nc()
    bass2jax.install_neuronx_cc_hook()
    partition_name = nc.partition_id_tensor.name if nc.partition_id_tensor else None
    in_names, out_names, out_avals = [], [], []
    for alloc in nc.m.functions[0].allocations:
        if not isinstance(alloc, mybir.MemoryLocationSet):
            continue
        name = alloc.memorylocations[0].name
        if alloc.kind == "ExternalInput":
            if name != partition_name:
                in_names.append(name)
        elif alloc.kind == "ExternalOutput":
            shape = tuple(alloc.tensor_shape)
            dtype = mybir.dt.np(alloc.dtype)
            out_names.append(name)
            out_avals.append(jax.core.ShapedArray(shape, dtype))
    n_params = len(in_names)
    n_outs = len(out_avals)
    in_names_all = in_names + out_names
    if partition_name is not None:
        in_names_all = in_names_all + [partition_name]

    def _body(*args):
        operands = list(args)
        if partition_name is not None:
            operands.append(bass2jax.partition_id_tensor())
        return tuple(
            bass2jax._bass_exec_p.bind(
                *operands,
                out_avals=tuple(out_avals),
                in_names=tuple(in_names_all),
                out_names=tuple(out_names),
                lowering_input_output_aliases=(),
                sim_require_finite=True,
                sim_require_nnan=True,
                nc=nc,
            )
        )

    devices = jax.devices()[:8]
    mesh = Mesh(np.asarray(devices), ("core",))
    in_specs = (PartitionSpec("core"),) * (n_params + n_outs)
    out_specs = (PartitionSpec("core"),) * len(out_names)
    donate = tuple(range(n_params, n_params + n_outs))
    sharded = jax.jit(
        shard_map(
            _body, mesh=mesh, in_specs=in_specs, out_specs=out_specs, check_rep=False
        ),
        donate_argnums=donate,
        keep_unused=True,
    )
    shard = NamedSharding(mesh, PartitionSpec("core"))

    # Donated output buffers are created on-device (a trivial jitted zeros
    # program) instead of being uploaded from the host every call.
    zeros_fns = [
        jax.jit(
            lambda aval=aval: jnp.zeros((8 * aval.shape[0], *aval.shape[1:]), aval.dtype),
            out_shardings=shard,
        )
        for aval in out_avals
    ]

    def run(in_maps):
        per_core = [[np.asarray(m[nm]) for nm in in_names] for m in in_maps]
        concat_in = [
            np.concatenate([per_core[c][i] for c in range(8)], axis=0)
            for i in range(n_params)
        ]
        dev_zeros = [fn() for fn in zeros_fns]
        out_arrs = sharded(*concat_in, *dev_zeros)
        return [
            {
                name: np.asarray(out_arrs[i]).reshape(8, *out_avals[i].shape)[c]
                for i, name in enumerate(out_names)
            }
            for c in range(8)
        ]

    _RUNNER = run
    return _RUNNER


def _make_in_maps(inputs):
    x = np.asarray(inputs["x"], dtype=np.float32)
    QK16 = np.asarray(inputs["QK"], dtype=np.float32).astype(NPBF16)
    VO16 = np.asarray(inputs["VO"], dtype=np.float32).astype(NPBF16)
    xb16 = [np.ascontiguousarray(x[b].astype(NPBF16)) for b in range(B)]
    in_maps = []
    for c in range(8):
        b, h = divmod(c, 2)
        if h == 0:
            xc = xb16[b]
        else:
            # Rotate rows so this core's queries are rows 0..TQ-1; the key
            # order is permuted identically in S and AV, which softmax and
            # the attention average are invariant to.
            xc = np.concatenate([xb16[b][TQ:], xb16[b][:TQ]], axis=0)
        xTc = np.ascontiguousarray(xc.T)
        in_maps.append({"x": xc, "xT": xTc, "QK": QK16, "VO": VO16})
    return in_maps


def kernel(x, QK, VO):
    in_maps = _make_in_maps({"x": x, "QK": QK, "VO": VO})
    results = _get_runner()(in_maps)
    out = np.empty((B, T, D), dtype=np.float32)
    for c in range(8):
        b, h = divmod(c, 2)
        out[b, h * TQ : (h + 1) * TQ, :] = results[c]["out"].astype(np.float32)
    return out


# revision 34
# speedup vs baseline: 1.1984x; 1.0816x over previous
"""Trainium2 Bass kernel for nn_MergedLinearFormer.

Computes out = softmax((x@QK)@x^T / sqrt(D)) @ x @ VO for x:[B,T,D].

Sharding: 8 cores; core c handles batch b=c//2, query half h=c%2 (2048
queries each). The host uploads one bf16 copy of x[b] per core, with the
rows rotated so this core's queries are rows 0..TQ-1 (key order is a
permutation, which softmax+AV are invariant to), plus the full 8 MB
transpose x^T (pre-transposed on the host; the on-chip xbar runs at only
~80 GB/s, so host transposition keeps the whole input stream on the fast
plain-DMA path). x^T stays SBUF-resident so the S-phase reads SBUF
directly and needs no per-chunk DMA.

Inside a core, everything is computed with the score matrix TRANSPOSED
(keys on PSUM partitions, queries on the free axis) so no on-chip
transposes are needed anywhere:

  phase 1:  xQK^T[e, q]   = QK^T @ xq^T          (lhsT=QK,  rhs=xT cols)
  S-phase:  S^T[u, q]     = x @ xQK^T             (lhsT=xT,  rhs=xQK^T)
            P^T[u, q]     = exp(S^T / sqrt(D))    (no max subtraction:
                             scores are ~N(0,1), exp can't overflow)
            colsum[q]    += P^T                   (DVE partial sums)
            den_j[q, 1]   = colsum_j^T @ ones     (tiny matmuls put the
                             denominators on partitions, no scatter DMA)
  AV-phase: av^T[d, q]    = x^T @ P^T             (lhsT=x,   rhs=P^T)
  OUT:      out[q, e]     = (av^T)^T @ VO         (lhsT=av^T, rhs=VO)
            out[q, e]    *= 1/den[q]

Startup is tuned around two HW facts seen in the trace: (a) the PE
clock ramps (0.65->1.2->2.4 GHz over the first ~3us of a busy streak),
so a handful of warm-up matmuls on memset scratch run while the first
input DMAs are still in flight; (b) phase 1 emitted et-major can only
finish its first PSUM tile after ALL of QK (2 MB) has landed, which
stalled the PE ~3.5us. Phase 1 for query block 0 is therefore emitted
kt-MAJOR across all 8 PSUM banks: round kt needs only one 384 KB
(QK-tile, xT-piece) pair, which DMA delivers faster than the PE
consumes it. The PSUM->SBUF copies are interleaved into the last round
(alternating ACT/DVE) so the S-phase can chase them tile by tile.

Loads ride the two HWDGE queues in strict order of first use (QK on
the ACT engine's queue; everything else on sync's — splitting the xT
stream across both queues was tried and starves the urgent pieces).
The drain end is handled with care: a HWDGE queue that has gone idle
takes ~1.5us to dispatch a fresh DMA (wake + descriptor generation),
so the last 512-col OUT chain is split into two 256-col chains whose
stores land on alternating queues — the first store's dispatch hides
under the second chain's matmuls.

All matmul operands are bf16 (PE streams 1 column/cycle regardless of
dtype, so bf16 halves DMA/SBUF at no PE cost); accumulation is fp32 in
PSUM; output bf16 (converted to fp32 on the host).
"""

import numpy as np
import ml_dtypes

import concourse.mybir as mybir
import concourse.tile as tile
from concourse import bacc

P = 128
B, T, D = 4, 4096, 1024
TQ = T // 2          # queries per core
CH = 512             # query-chunk width
ET = D // P          # 8 tiles along the model dim
UT = T // P          # 32 tiles along the key dim
UG = T // 512        # 8 key groups of 512
CHUNKS = TQ // CH    # 4
JT = CH // P         # 4 query tiles per chunk
SCALE = 1.0 / np.sqrt(D)

BF16 = mybir.dt.bfloat16
F32 = mybir.dt.float32
NPBF16 = ml_dtypes.bfloat16


def _build():
    nc = bacc.Bacc()
    x_ = nc.dram_tensor("x", [T, D], BF16, kind="ExternalInput")
    xTd = nc.dram_tensor("xT", [D, T], BF16, kind="ExternalInput")
    QK = nc.dram_tensor("QK", [D, D], BF16, kind="ExternalInput")
    VO = nc.dram_tensor("VO", [D, D], BF16, kind="ExternalInput")
    out = nc.dram_tensor("out", [TQ, D], BF16, kind="ExternalOutput")

    x_r = x_.rearrange("(uo p) d -> p uo d", p=P)       # [128, 32, 1024]
    xT_r = xTd.rearrange("(eo p) u -> p eo u", p=P)     # [128, 8, 4096]
    QK_r = QK.rearrange("(ko p) e -> p ko e", p=P)      # [128, 8, 1024]
    VO_r = VO.rearrange("(ko p) e -> p ko e", p=P)      # [128, 8, 1024]

    with tile.TileContext(nc) as tc:
        with (
            tc.tile_pool(name="resident", bufs=1) as resident,
            tc.tile_pool(name="consts", bufs=1) as consts,
            tc.tile_pool(name="ptpool", bufs=UT + 2) as ptpool,
            tc.tile_pool(name="xpan_pool", bufs=4) as xpan_pool,
            tc.tile_pool(name="avpool", bufs=2) as avpool,
            tc.tile_pool(name="outpool", bufs=2) as outpool,
            tc.tile_pool(name="small", bufs=2) as small,
            tc.tile_pool(name="ps_acc", bufs=3, space="PSUM") as ps_acc_pool,
            tc.tile_pool(name="ps_sums", bufs=1, space="PSUM") as ps_sums_pool,
            tc.tile_pool(name="ps_av", bufs=2, space="PSUM") as ps_av_pool,
            tc.tile_pool(name="ps_o", bufs=2, space="PSUM") as ps_o_pool,
        ):
            # SBUF-resident transposed x and xQK^T.
            xT = resident.tile([P, ET, T], BF16)      # 8 MB
            xqkt = resident.tile([P, ET, TQ], BF16)   # 4 MB
            qk_sb = resident.tile([P, ET, D], BF16)   # 2 MB
            scratch = consts.tile([P, 512], BF16)     # PE warm-up fodder
            ones_sb = consts.tile([P, 1], F32)
            vo_sb = consts.tile([P, ET, D], BF16)

            nc.gpsimd.memset(scratch, 0.0)
            nc.vector.memset(ones_sb, 1.0)

            # ---- loads, in order of first use ----
            # Critical pairs first: phase-1 round kt consumes exactly
            # (qk tile kt, xT[:, kt, 0:512]).  QK (just 8 dispatches)
            # goes via the scalar engine's queue, the whole xT stream
            # via sync's, in strict order of first use: query cols
            # 512:2048 (S ut4-15 consume them from ~30us), then the key
            # half (S ut16+ from ~50us), then VO (OUT phase, >150us).
            # Two hard-won rules: (1) splitting ONE stream across both
            # queues starves the urgent pieces; (2) the big dispatch
            # backlog must live on SYNC — a long backlog on the
            # scalar/ACT engine blocks at ring-full and stalls the
            # phase-1 copies and exps behind it (+44us).
            for kt in range(ET):
                nc.scalar.dma_start(qk_sb[:, kt, :], QK_r[:, kt, :])
                nc.sync.dma_start(xT[:, kt, 0:512], xT_r[:, kt, 0:512])
            for ug in range(1, UG):
                for kt in range(ET):
                    nc.sync.dma_start(
                        xT[:, kt, ug * 512 : (ug + 1) * 512],
                        xT_r[:, kt, ug * 512 : (ug + 1) * 512],
                    )
            nc.sync.dma_start(vo_sb, VO_r)

            # ---- PE clock warm-up ----
            # The tensor engine ramps 0.65 -> 1.2 -> 2.4 GHz over the
            # first ~3us of a busy streak.  Burn the DMA-bound startup
            # window on scratch matmuls so the real phase-1 rounds run
            # at full clock from their first instruction.
            warm_ps = ps_o_pool.tile([P, 512], F32, name="o_ps")
            for _ in range(8):
                nc.tensor.matmul(
                    warm_ps, scratch[:, 0:P], scratch, start=True, stop=True
                )

            # ---- phase 1 for query block 0: kt-major over 8 banks ----
            # Each round kt touches one (qk[kt], xT[:,kt,0:512]) pair,
            # so the PE never waits for the whole 2 MB of QK.  Copies
            # to xqkt ride the last round, alternating ACT/DVE, in et
            # order -- exactly the order the S-phase consumes them.
            ps8 = (
                [ps_acc_pool.tile([P, 512], F32, name="acc_ps") for _ in range(3)]
                + [ps_sums_pool.tile([P, 512], F32, name="den_ps")]
                + [ps_av_pool.tile([P, 512], F32, name="av_ps") for _ in range(2)]
                + [ps_o_pool.tile([P, 512], F32, name="o_ps") for _ in range(2)]
            )
            for kt in range(ET):
                for et in range(ET):
                    nc.tensor.matmul(
                        ps8[et],
                        qk_sb[:, kt, et * P : (et + 1) * P],
                        xT[:, kt, 0:512],
                        start=(kt == 0),
                        stop=(kt == ET - 1),
                    )
                    if kt == ET - 1:
                        if et % 2 == 0:
                            nc.scalar.copy(xqkt[:, et, 0:512], ps8[et])
                        else:
                            nc.vector.tensor_copy(xqkt[:, et, 0:512], ps8[et])

            # ---- phase 1 for query blocks 1-3 (steady state) ----
            def ph1_nq(nq):
                for et in range(ET):
                    ps = ps_acc_pool.tile([P, 512], F32, name="acc_ps")
                    for kt in range(ET):
                        nc.tensor.matmul(
                            ps,
                            qk_sb[:, kt, et * P : (et + 1) * P],
                            xT[:, kt, nq * 512 : (nq + 1) * 512],
                            start=(kt == 0),
                            stop=(kt == ET - 1),
                        )
                    nc.scalar.copy(xqkt[:, et, nq * 512 : (nq + 1) * 512], ps)

            # ---- S-phase tile: S^T, exp, denominator accumulation ----
            def s_tile(ut, q0, colsum, pts):
                s_ps = ps_acc_pool.tile([P, CH], F32, name="acc_ps")
                for kt in range(ET):
                    nc.tensor.matmul(
                        s_ps,
                        xT[:, kt, ut * P : (ut + 1) * P],
                        xqkt[:, kt, q0 : q0 + CH],
                        start=(kt == 0),
                        stop=(kt == ET - 1),
                    )
                pt = ptpool.tile([P, CH], BF16, name="pt")
                nc.scalar.activation(
                    pt, s_ps, mybir.ActivationFunctionType.Exp, scale=SCALE
                )
                if ut == 0:
                    nc.vector.tensor_copy(colsum, pt)
                else:
                    nc.vector.tensor_add(colsum, colsum, pt)
                pts.append(pt)

            for c in range(CHUNKS):
                q0 = c * CH
                colsum = small.tile([P, CH], F32, name="colsum")
                pts = []
                for ut in range(UT):
                    s_tile(ut, q0, colsum, pts)
                if c == 0:
                    # The remaining phase-1 blocks run here, after
                    # chunk 0's S-phase: by now the whole input stream
                    # is resident, and chunk 1's S-phase needs xqkt
                    # nq1 only after chunk 0's AV+OUT (~55us away).
                    for nq in range(1, CHUNKS):
                        ph1_nq(nq)

                # ---- AV-phase: av^T[d, q] ----
                # x panels stream in 512 KB halves so a late DMA only
                # stalls 16 of the 32 accumulation matmuls.
                av_sb = avpool.tile([P, ET, CH], BF16, name="av_sb")
                UH = UT // 2
                for dt in range(ET):
                    av_ps = ps_av_pool.tile([P, CH], F32, name="av_ps")
                    for uh in range(2):
                        x_pan = xpan_pool.tile([P, UH, P], BF16, name="x_pan")
                        nc.sync.dma_start(
                            x_pan,
                            x_r[:, uh * UH : (uh + 1) * UH, dt * P : (dt + 1) * P],
                        )
                        for ui in range(UH):
                            ut = uh * UH + ui
                            nc.tensor.matmul(
                                av_ps,
                                x_pan[:, ui, :],
                                pts[ut],
                                start=(ut == 0),
                                stop=(ut == UT - 1),
                            )
                    nc.vector.tensor_copy(av_sb[:, dt, :], av_ps)

                # ---- denominators -> per-partition reciprocals ----
                # Emitted AFTER the AV matmuls: these tiny matmuls wait on
                # the ACT exp of the last S tile (via colsum), and the PE
                # queue is strictly in-order — placed between S and AV
                # they stall the AV start ~1 us per chunk. Here they slot
                # between AV and OUT, and the reciprocal easily beats
                # OUT's first normalization. Four matmuls write disjoint
                # columns of one PSUM bank (start only on the first:
                # later writes land on never-written elements, so they
                # overwrite, not add).
                den_ps = ps_sums_pool.tile([P, JT], F32, name="den_ps")
                for j in range(JT):
                    nc.tensor.matmul(
                        den_ps[:, j : j + 1],
                        colsum[:, j * P : (j + 1) * P],
                        ones_sb,
                        start=(j == 0),
                        stop=(j == JT - 1),
                    )
                r_sb = small.tile([P, JT], F32, name="r_sb")
                nc.vector.reciprocal(r_sb, den_ps)

                # ---- OUT: (av^T)^T @ VO, normalized ----
                store_flip = 0
                for j in range(JT):
                    out_sb = outpool.tile([P, D], BF16, name="out_sb")
                    rows = slice(q0 + j * P, q0 + (j + 1) * P)
                    for eh in range(2):
                        last = c == CHUNKS - 1 and j == JT - 1 and eh == 1
                        if last:
                            # Final piece: run it as two 256-col chains.
                            # The first half's normalize+store (and the
                            # ~1.5us queue-wake latency of its dispatch)
                            # overlap the second half's matmuls, so only
                            # a 256-col store remains after the last
                            # matmul — on the scalar queue, which is
                            # still hot from the earlier stores.
                            for qh in range(2):
                                o_ps = ps_o_pool.tile([P, 256], F32, name="o_ps")
                                for dt in range(ET):
                                    nc.tensor.matmul(
                                        o_ps,
                                        av_sb[:, dt, j * P : (j + 1) * P],
                                        vo_sb[
                                            :,
                                            dt,
                                            512 + qh * 256 : 512 + (qh + 1) * 256,
                                        ],
                                        start=(dt == 0),
                                        stop=(dt == ET - 1),
                                    )
                                sl = slice(512 + qh * 256, 512 + (qh + 1) * 256)
                                nc.vector.tensor_scalar_mul(
                                    out_sb[:, sl], o_ps, r_sb[:, j : j + 1]
                                )
                                if qh == 0:
                                    nc.scalar.dma_start(
                                        out[rows, sl], out_sb[:, sl]
                                    )
                                else:
                                    # Very last store: split by rows so
                                    # each queue generates only 64
                                    # descriptors (~11ns/row) in
                                    # parallel.
                                    r0 = q0 + j * P
                                    nc.sync.dma_start(
                                        out[r0 : r0 + 64, sl],
                                        out_sb[0:64, sl],
                                    )
                                    nc.scalar.dma_start(
                                        out[r0 + 64 : r0 + 128, sl],
                                        out_sb[64:128, sl],
                                    )
                                store_flip += 1
                            continue
                        o_ps = ps_o_pool.tile([P, 512], F32, name="o_ps")
                        for dt in range(ET):
                            nc.tensor.matmul(
                                o_ps,
                                av_sb[:, dt, j * P : (j + 1) * P],
                                vo_sb[:, dt, eh * 512 : (eh + 1) * 512],
                                start=(dt == 0),
                                stop=(dt == ET - 1),
                            )
                        nc.vector.tensor_scalar_mul(
                            out_sb[:, eh * 512 : (eh + 1) * 512],
                            o_ps,
                            r_sb[:, j : j + 1],
                        )
                        if c == CHUNKS - 1:
                            # Last chunk: store each half as soon as its
                            # normalization lands, alternating queues so
                            # neither backs up at the drain.
                            eng = nc.sync if store_flip % 2 == 0 else nc.scalar
                            store_flip += 1
                            eng.dma_start(
                                out[rows, eh * 512 : (eh + 1) * 512],
                                out_sb[:, eh * 512 : (eh + 1) * 512],
                            )
                    if c != CHUNKS - 1:
                        # Scalar engine's queue: keeps the sync queue
                        # exclusively on x-panel prefetch, so stores
                        # never delay the next chunk's AV data.
                        nc.scalar.dma_start(out[rows, :], out_sb)

    nc.compile()
    return nc


_NC = None


def _get_nc():
    global _NC
    if _NC is None:
        _NC = _build()
    return _NC


_RUNNER = None


def _get_runner():
    """Cached jitted 8-core SPMD executor (avoids re-tracing per call)."""
    global _RUNNER
    if _RUNNER is not None:
        return _RUNNER

    import jax
    import jax.numpy as jnp
    from jax.sharding import Mesh, NamedSharding, PartitionSpec
    from jax.experimental.shard_map import shard_map
    from concourse import bass2jax

    nc = _get_nc()
    bass2jax.install_neuronx_cc_hook()
    partition_name = nc.partition_id_tensor.name if nc.partition_id_tensor else None
    in_names, out_names, out_avals = [], [], []
    for alloc in nc.m.functions[0].allocations:
        if not isinstance(alloc, mybir.MemoryLocationSet):
            continue
        name = alloc.memorylocations[0].name
        if alloc.kind == "ExternalInput":
            if name != partition_name:
                in_names.append(name)
        elif alloc.kind == "ExternalOutput":
            shape = tuple(alloc.tensor_shape)
            dtype = mybir.dt.np(alloc.dtype)
            out_names.append(name)
            out_avals.append(jax.core.ShapedArray(shape, dtype))
    n_params = len(in_names)
    n_outs = len(out_avals)
    in_names_all = in_names + out_names
    if partition_name is not None:
        in_names_all = in_names_all + [partition_name]

    def _body(*args):
        operands = list(args)
        if partition_name is not None:
            operands.append(bass2jax.partition_id_tensor())
        return tuple(
            bass2jax._bass_exec_p.bind(
                *operands,
                out_avals=tuple(out_avals),
                in_names=tuple(in_names_all),
                out_names=tuple(out_names),
                lowering_input_output_aliases=(),
                sim_require_finite=True,
                sim_require_nnan=True,
                nc=nc,
            )
        )

    devices = jax.devices()[:8]
    mesh = Mesh(np.asarray(devices), ("core",))
    in_specs = (PartitionSpec("core"),) * (n_params + n_outs)
    out_specs = (PartitionSpec("core"),) * len(out_names)
    donate = tuple(range(n_params, n_params + n_outs))
    sharded = jax.jit(
        shard_map(
            _body, mesh=mesh, in_specs=in_specs, out_specs=out_specs, check_rep=False
        ),
        donate_argnums=donate,
        keep_unused=True,
    )
    shard = NamedSharding(mesh, PartitionSpec("core"))

    # Donated output buffers are created on-device (a trivial jitted zeros
    # program) instead of being uploaded from the host every call.
    zeros_fns = [
        jax.jit(
            lambda aval=aval: jnp.zeros((8 * aval.shape[0], *aval.shape[1:]), aval.dtype),
            out_shardings=shard,
        )
        for aval in out_avals
    ]

    def run(in_maps):
        per_core = [[np.asarray(m[nm]) for nm in in_names] for m in in_maps]
        concat_in = [
            np.concatenate([per_core[c][i] for c in range(8)], axis=0)
            for i in range(n_params)
        ]
        dev_zeros = [fn() for fn in zeros_fns]
        out_arrs = sharded(*concat_in, *dev_zeros)
        return [
            {
                name: np.asarray(out_arrs[i]).reshape(8, *out_avals[i].shape)[c]
                for i, name in enumerate(out_names)
            }
            for c in range(8)
        ]

    _RUNNER = run
    return _RUNNER


def _make_in_maps(inputs):
    x = np.asarray(inputs["x"], dtype=np.float32)
    QK16 = np.asarray(inputs["QK"], dtype=np.float32).astype(NPBF16)
    VO16 = np.asarray(inputs["VO"], dtype=np.float32).astype(NPBF16)
    xb16 = [np.ascontiguousarray(x[b].astype(NPBF16)) for b in range(B)]
    in_maps = []
    for c in range(8):
        b, h = divmod(c, 2)
        if h == 0:
            xc = xb16[b]
        else:
            # Rotate rows so this core's queries are rows 0..TQ-1; the key
            # order is permuted identically in S and AV, which softmax and
            # the attention average are invariant to.
            xc = np.concatenate([xb16[b][TQ:], xb16[b][:TQ]], axis=0)
        xTc = np.ascontiguousarray(xc.T)
        in_maps.append({"x": xc, "xT": xTc, "QK": QK16, "VO": VO16})
    return in_maps


def kernel(x, QK, VO):
    in_maps = _make_in_maps({"x": x, "QK": QK, "VO": VO})
    results = _get_runner()(in_maps)
    out = np.empty((B, T, D), dtype=np.float32)
    for c in range(8):
        b, h = divmod(c, 2)
        out[b, h * TQ : (h + 1) * TQ, :] = results[c]["out"].astype(np.float32)
    return out


# revision 35
# speedup vs baseline: 1.2000x; 1.0014x over previous
"""Trainium2 Bass kernel for nn_MergedLinearFormer.

Computes out = softmax((x@QK)@x^T / sqrt(D)) @ x @ VO for x:[B,T,D].

Sharding: 8 cores; core c handles batch b=c//2, query half h=c%2 (2048
queries each). The host uploads one bf16 copy of x[b] per core, with the
rows rotated so this core's queries are rows 0..TQ-1 (key order is a
permutation, which softmax+AV are invariant to), plus the full 8 MB
transpose x^T (pre-transposed on the host; the on-chip xbar runs at only
~80 GB/s, so host transposition keeps the whole input stream on the fast
plain-DMA path). x^T stays SBUF-resident so the S-phase reads SBUF
directly and needs no per-chunk DMA.

Inside a core, everything is computed with the score matrix TRANSPOSED
(keys on PSUM partitions, queries on the free axis) so no on-chip
transposes are needed anywhere:

  phase 1:  xQK^T[e, q]   = QK^T @ xq^T          (lhsT=QK,  rhs=xT cols)
  S-phase:  S^T[u, q]     = x @ xQK^T             (lhsT=xT,  rhs=xQK^T)
            P^T[u, q]     = exp(S^T / sqrt(D))    (no max subtraction:
                             scores are ~N(0,1), exp can't overflow)
            colsum[q]    += P^T                   (DVE partial sums)
            den_j[q, 1]   = colsum_j^T @ ones     (tiny matmuls put the
                             denominators on partitions, no scatter DMA)
  AV-phase: av^T[d, q]    = x^T @ P^T             (lhsT=x,   rhs=P^T)
  OUT:      out[q, e]     = (av^T)^T @ VO         (lhsT=av^T, rhs=VO)
            out[q, e]    *= 1/den[q]

Startup is tuned around two HW facts seen in the trace: (a) the PE
clock ramps (0.65->1.2->2.4 GHz over the first ~3us of a busy streak),
so a handful of warm-up matmuls on memset scratch run while the first
input DMAs are still in flight; (b) phase 1 emitted et-major can only
finish its first PSUM tile after ALL of QK (2 MB) has landed, which
stalled the PE ~3.5us. Phase 1 for query block 0 is therefore emitted
kt-MAJOR across all 8 PSUM banks: round kt needs only one 384 KB
(QK-tile, xT-piece) pair, which DMA delivers faster than the PE
consumes it. The PSUM->SBUF copies are interleaved into the last round
(alternating ACT/DVE) so the S-phase can chase them tile by tile.

Loads ride the two HWDGE queues in strict order of first use (QK on
the ACT engine's queue; everything else on sync's — splitting the xT
stream across both queues was tried and starves the urgent pieces).
The drain end is handled with care: a HWDGE queue that has gone idle
takes ~1.5us to dispatch a fresh DMA (wake + descriptor generation),
so the last 512-col OUT chain is split into two 256-col chains whose
stores land on alternating queues — the first store's dispatch hides
under the second chain's matmuls.

All matmul operands are bf16 (PE streams 1 column/cycle regardless of
dtype, so bf16 halves DMA/SBUF at no PE cost); accumulation is fp32 in
PSUM; output bf16 (converted to fp32 on the host).
"""

import numpy as np
import ml_dtypes

import concourse.mybir as mybir
import concourse.tile as tile
from concourse import bacc

P = 128
B, T, D = 4, 4096, 1024
TQ = T // 2          # queries per core
CH = 512             # query-chunk width
ET = D // P          # 8 tiles along the model dim
UT = T // P          # 32 tiles along the key dim
UG = T // 512        # 8 key groups of 512
CHUNKS = TQ // CH    # 4
JT = CH // P         # 4 query tiles per chunk
SCALE = 1.0 / np.sqrt(D)

BF16 = mybir.dt.bfloat16
F32 = mybir.dt.float32
NPBF16 = ml_dtypes.bfloat16


def _build():
    nc = bacc.Bacc()
    x_ = nc.dram_tensor("x", [T, D], BF16, kind="ExternalInput")
    xTd = nc.dram_tensor("xT", [D, T], BF16, kind="ExternalInput")
    QK = nc.dram_tensor("QK", [D, D], BF16, kind="ExternalInput")
    VO = nc.dram_tensor("VO", [D, D], BF16, kind="ExternalInput")
    out = nc.dram_tensor("out", [TQ, D], BF16, kind="ExternalOutput")

    x_r = x_.rearrange("(uo p) d -> p uo d", p=P)       # [128, 32, 1024]
    xT_r = xTd.rearrange("(eo p) u -> p eo u", p=P)     # [128, 8, 4096]
    QK_r = QK.rearrange("(ko p) e -> p ko e", p=P)      # [128, 8, 1024]
    VO_r = VO.rearrange("(ko p) e -> p ko e", p=P)      # [128, 8, 1024]

    with tile.TileContext(nc) as tc:
        with (
            tc.tile_pool(name="resident", bufs=1) as resident,
            tc.tile_pool(name="consts", bufs=1) as consts,
            tc.tile_pool(name="ptpool", bufs=UT + 2) as ptpool,
            tc.tile_pool(name="xpan_pool", bufs=4) as xpan_pool,
            tc.tile_pool(name="avpool", bufs=2) as avpool,
            tc.tile_pool(name="outpool", bufs=2) as outpool,
            tc.tile_pool(name="small", bufs=2) as small,
            tc.tile_pool(name="ps_acc", bufs=3, space="PSUM") as ps_acc_pool,
            tc.tile_pool(name="ps_sums", bufs=1, space="PSUM") as ps_sums_pool,
            tc.tile_pool(name="ps_av", bufs=2, space="PSUM") as ps_av_pool,
            tc.tile_pool(name="ps_o", bufs=2, space="PSUM") as ps_o_pool,
        ):
            # SBUF-resident transposed x and xQK^T.
            xT = resident.tile([P, ET, T], BF16)      # 8 MB
            xqkt = resident.tile([P, ET, TQ], BF16)   # 4 MB
            qk_sb = resident.tile([P, ET, D], BF16)   # 2 MB
            scratch = consts.tile([P, 512], BF16)     # PE warm-up fodder
            ones_sb = consts.tile([P, 1], F32)
            vo_sb = consts.tile([P, ET, D], BF16)

            nc.gpsimd.memset(scratch, 0.0)
            nc.vector.memset(ones_sb, 1.0)

            # ---- loads, in order of first use ----
            # Critical pairs first: phase-1 round kt consumes exactly
            # (qk tile kt, xT[:, kt, 0:512]).  QK (just 8 dispatches)
            # goes via the scalar engine's queue, the whole xT stream
            # via sync's, in strict order of first use: query cols
            # 512:2048 (S ut4-15 consume them from ~30us), then the key
            # half (S ut16+ from ~50us), then VO (OUT phase, >150us).
            # Two hard-won rules: (1) splitting ONE stream across both
            # queues starves the urgent pieces; (2) the big dispatch
            # backlog must live on SYNC — a long backlog on the
            # scalar/ACT engine blocks at ring-full and stalls the
            # phase-1 copies and exps behind it (+44us).
            for kt in range(ET):
                nc.scalar.dma_start(qk_sb[:, kt, :], QK_r[:, kt, :])
                nc.sync.dma_start(xT[:, kt, 0:512], xT_r[:, kt, 0:512])
            for ug in range(1, UG):
                for kt in range(ET):
                    nc.sync.dma_start(
                        xT[:, kt, ug * 512 : (ug + 1) * 512],
                        xT_r[:, kt, ug * 512 : (ug + 1) * 512],
                    )
            nc.sync.dma_start(vo_sb, VO_r)

            # ---- PE clock warm-up ----
            # The tensor engine ramps 0.65 -> 1.2 -> 2.4 GHz over the
            # first ~3us of a busy streak.  Burn the DMA-bound startup
            # window on scratch matmuls so the real phase-1 rounds run
            # at full clock from their first instruction.
            warm_ps = ps_o_pool.tile([P, 512], F32, name="o_ps")
            for _ in range(9):
                nc.tensor.matmul(
                    warm_ps, scratch[:, 0:P], scratch, start=True, stop=True
                )

            # ---- phase 1 for query block 0: kt-major over 8 banks ----
            # Each round kt touches one (qk[kt], xT[:,kt,0:512]) pair,
            # so the PE never waits for the whole 2 MB of QK.  Copies
            # to xqkt ride the last round, alternating ACT/DVE, in et
            # order -- exactly the order the S-phase consumes them.
            ps8 = (
                [ps_acc_pool.tile([P, 512], F32, name="acc_ps") for _ in range(3)]
                + [ps_sums_pool.tile([P, 512], F32, name="den_ps")]
                + [ps_av_pool.tile([P, 512], F32, name="av_ps") for _ in range(2)]
                + [ps_o_pool.tile([P, 512], F32, name="o_ps") for _ in range(2)]
            )
            for kt in range(ET):
                for et in range(ET):
                    nc.tensor.matmul(
                        ps8[et],
                        qk_sb[:, kt, et * P : (et + 1) * P],
                        xT[:, kt, 0:512],
                        start=(kt == 0),
                        stop=(kt == ET - 1),
                    )
                    if kt == ET - 1:
                        if et % 2 == 0:
                            nc.scalar.copy(xqkt[:, et, 0:512], ps8[et])
                        else:
                            nc.vector.tensor_copy(xqkt[:, et, 0:512], ps8[et])

            # ---- phase 1 for query blocks 1-3 (steady state) ----
            def ph1_nq(nq):
                for et in range(ET):
                    ps = ps_acc_pool.tile([P, 512], F32, name="acc_ps")
                    for kt in range(ET):
                        nc.tensor.matmul(
                            ps,
                            qk_sb[:, kt, et * P : (et + 1) * P],
                            xT[:, kt, nq * 512 : (nq + 1) * 512],
                            start=(kt == 0),
                            stop=(kt == ET - 1),
                        )
                    nc.scalar.copy(xqkt[:, et, nq * 512 : (nq + 1) * 512], ps)

            # ---- S-phase tile: S^T, exp, denominator accumulation ----
            def s_tile(ut, q0, colsum, pts):
                s_ps = ps_acc_pool.tile([P, CH], F32, name="acc_ps")
                for kt in range(ET):
                    nc.tensor.matmul(
                        s_ps,
                        xT[:, kt, ut * P : (ut + 1) * P],
                        xqkt[:, kt, q0 : q0 + CH],
                        start=(kt == 0),
                        stop=(kt == ET - 1),
                    )
                pt = ptpool.tile([P, CH], BF16, name="pt")
                nc.scalar.activation(
                    pt, s_ps, mybir.ActivationFunctionType.Exp, scale=SCALE
                )
                if ut == 0:
                    nc.vector.tensor_copy(colsum, pt)
                else:
                    nc.vector.tensor_add(colsum, colsum, pt)
                pts.append(pt)

            for c in range(CHUNKS):
                q0 = c * CH
                colsum = small.tile([P, CH], F32, name="colsum")
                pts = []
                for ut in range(UT):
                    s_tile(ut, q0, colsum, pts)
                if c == 0:
                    # The remaining phase-1 blocks run here, after
                    # chunk 0's S-phase: by now the whole input stream
                    # is resident, and chunk 1's S-phase needs xqkt
                    # nq1 only after chunk 0's AV+OUT (~55us away).
                    for nq in range(1, CHUNKS):
                        ph1_nq(nq)

                # ---- AV-phase: av^T[d, q] ----
                # x panels stream in 512 KB halves so a late DMA only
                # stalls 16 of the 32 accumulation matmuls.
                av_sb = avpool.tile([P, ET, CH], BF16, name="av_sb")
                UH = UT // 2
                for dt in range(ET):
                    av_ps = ps_av_pool.tile([P, CH], F32, name="av_ps")
                    for uh in range(2):
                        x_pan = xpan_pool.tile([P, UH, P], BF16, name="x_pan")
                        nc.sync.dma_start(
                            x_pan,
                            x_r[:, uh * UH : (uh + 1) * UH, dt * P : (dt + 1) * P],
                        )
                        for ui in range(UH):
                            ut = uh * UH + ui
                            nc.tensor.matmul(
                                av_ps,
                                x_pan[:, ui, :],
                                pts[ut],
                                start=(ut == 0),
                                stop=(ut == UT - 1),
                            )
                    nc.vector.tensor_copy(av_sb[:, dt, :], av_ps)

                # ---- denominators -> per-partition reciprocals ----
                # Emitted AFTER the AV matmuls: these tiny matmuls wait on
                # the ACT exp of the last S tile (via colsum), and the PE
                # queue is strictly in-order — placed between S and AV
                # they stall the AV start ~1 us per chunk. Here they slot
                # between AV and OUT, and the reciprocal easily beats
                # OUT's first normalization. Four matmuls write disjoint
                # columns of one PSUM bank (start only on the first:
                # later writes land on never-written elements, so they
                # overwrite, not add).
                den_ps = ps_sums_pool.tile([P, JT], F32, name="den_ps")
                for j in range(JT):
                    nc.tensor.matmul(
                        den_ps[:, j : j + 1],
                        colsum[:, j * P : (j + 1) * P],
                        ones_sb,
                        start=(j == 0),
                        stop=(j == JT - 1),
                    )
                r_sb = small.tile([P, JT], F32, name="r_sb")
                nc.vector.reciprocal(r_sb, den_ps)

                # ---- OUT: (av^T)^T @ VO, normalized ----
                store_flip = 0
                for j in range(JT):
                    out_sb = outpool.tile([P, D], BF16, name="out_sb")
                    rows = slice(q0 + j * P, q0 + (j + 1) * P)
                    for eh in range(2):
                        last = c == CHUNKS - 1 and j == JT - 1 and eh == 1
                        if last:
                            # Final piece: run it as two 256-col chains.
                            # The first half's normalize+store (and the
                            # ~1.5us queue-wake latency of its dispatch)
                            # overlap the second half's matmuls, so only
                            # a 256-col store remains after the last
                            # matmul — on the scalar queue, which is
                            # still hot from the earlier stores.
                            for qh in range(2):
                                o_ps = ps_o_pool.tile([P, 256], F32, name="o_ps")
                                for dt in range(ET):
                                    nc.tensor.matmul(
                                        o_ps,
                                        av_sb[:, dt, j * P : (j + 1) * P],
                                        vo_sb[
                                            :,
                                            dt,
                                            512 + qh * 256 : 512 + (qh + 1) * 256,
                                        ],
                                        start=(dt == 0),
                                        stop=(dt == ET - 1),
                                    )
                                sl = slice(512 + qh * 256, 512 + (qh + 1) * 256)
                                nc.vector.tensor_scalar_mul(
                                    out_sb[:, sl], o_ps, r_sb[:, j : j + 1]
                                )
                                if qh == 0:
                                    nc.scalar.dma_start(
                                        out[rows, sl], out_sb[:, sl]
                                    )
                                else:
                                    # Very last store: split by rows so
                                    # each queue generates only 64
                                    # descriptors (~11ns/row) in
                                    # parallel.
                                    r0 = q0 + j * P
                                    nc.sync.dma_start(
                                        out[r0 : r0 + 64, sl],
                                        out_sb[0:64, sl],
                                    )
                                    nc.scalar.dma_start(
                                        out[r0 + 64 : r0 + 128, sl],
                                        out_sb[64:128, sl],
                                    )
                                store_flip += 1
                            continue
                        o_ps = ps_o_pool.tile([P, 512], F32, name="o_ps")
                        for dt in range(ET):
                            nc.tensor.matmul(
                                o_ps,
                                av_sb[:, dt, j * P : (j + 1) * P],
                                vo_sb[:, dt, eh * 512 : (eh + 1) * 512],
                                start=(dt == 0),
                                stop=(dt == ET - 1),
                            )
                        nc.vector.tensor_scalar_mul(
                            out_sb[:, eh * 512 : (eh + 1) * 512],
                            o_ps,
                            r_sb[:, j : j + 1],
                        )
                        if c == CHUNKS - 1:
                            # Last chunk: store each half as soon as its
                            # normalization lands, alternating queues so
                            # neither backs up at the drain.
                            eng = nc.sync if store_flip % 2 == 0 else nc.scalar
                            store_flip += 1
                            eng.dma_start(
                                out[rows, eh * 512 : (eh + 1) * 512],
                                out_sb[:, eh * 512 : (eh + 1) * 512],
                            )
                    if c != CHUNKS - 1:
                        # Scalar engine's queue: keeps the sync queue
                        # exclusively on x-panel prefetch, so stores
                        # never delay the next chunk's AV data.
                        nc.scalar.dma_start(out[rows, :], out_sb)

    nc.compile()
    return nc


_NC = None


def _get_nc():
    global _NC
    if _NC is None:
        _NC = _build()
    return _NC


_RUNNER = None


def _get_runner():
    """Cached jitted 8-core SPMD executor (avoids re-tracing per call)."""
    global _RUNNER
    if _RUNNER is not None:
        return _RUNNER

    import jax
    import jax.numpy as jnp
    from jax.sharding import Mesh, NamedSharding, PartitionSpec
    from jax.experimental.shard_map import shard_map
    from concourse import bass2jax

    nc = _get_nc()
    bass2jax.install_neuronx_cc_hook()
    partition_name = nc.partition_id_tensor.name if nc.partition_id_tensor else None
    in_names, out_names, out_avals = [], [], []
    for alloc in nc.m.functions[0].allocations:
        if not isinstance(alloc, mybir.MemoryLocationSet):
            continue
        name = alloc.memorylocations[0].name
        if alloc.kind == "ExternalInput":
            if name != partition_name:
                in_names.append(name)
        elif alloc.kind == "ExternalOutput":
            shape = tuple(alloc.tensor_shape)
            dtype = mybir.dt.np(alloc.dtype)
            out_names.append(name)
            out_avals.append(jax.core.ShapedArray(shape, dtype))
    n_params = len(in_names)
    n_outs = len(out_avals)
    in_names_all = in_names + out_names
    if partition_name is not None:
        in_names_all = in_names_all + [partition_name]

    def _body(*args):
        operands = list(args)
        if partition_name is not None:
            operands.append(bass2jax.partition_id_tensor())
        return tuple(
            bass2jax._bass_exec_p.bind(
                *operands,
                out_avals=tuple(out_avals),
                in_names=tuple(in_names_all),
                out_names=tuple(out_names),
                lowering_input_output_aliases=(),
                sim_require_finite=True,
                sim_require_nnan=True,
                nc=nc,
            )
        )

    devices = jax.devices()[:8]
    mesh = Mesh(np.asarray(devices), ("core",))
    in_specs = (PartitionSpec("core"),) * (n_params + n_outs)
    out_specs = (PartitionSpec("core"),) * len(out_names)
    donate = tuple(range(n_params, n_params + n_outs))
    sharded = jax.jit(
        shard_map(
            _body, mesh=mesh, in_specs=in_specs, out_specs=out_specs, check_rep=False
        ),
        donate_argnums=donate,
        keep_unused=True,
    )
    shard = NamedSharding(mesh, PartitionSpec("core"))

    # Donated output buffers are created on-device (a trivial jitted zeros
    # program) instead of being uploaded from the host every call.
    zeros_fns = [
        jax.jit(
            lambda aval=aval: jnp.zeros((8 * aval.shape[0], *aval.shape[1:]), aval.dtype),
            out_shardings=shard,
        )
        for aval in out_avals
    ]

    def run(in_maps):
        per_core = [[np.asarray(m[nm]) for nm in in_names] for m in in_maps]
        concat_in = [
            np.concatenate([per_core[c][i] for c in range(8)], axis=0)
            for i in range(n_params)
        ]
        dev_zeros = [fn() for fn in zeros_fns]
        out_arrs = sharded(*concat_in, *dev_zeros)
        return [
            {
                name: np.asarray(out_arrs[i]).reshape(8, *out_avals[i].shape)[c]
                for i, name in enumerate(out_names)
            }
            for c in range(8)
        ]

    _RUNNER = run
    return _RUNNER


def _make_in_maps(inputs):
    x = np.asarray(inputs["x"], dtype=np.float32)
    QK16 = np.asarray(inputs["QK"], dtype=np.float32).astype(NPBF16)
    VO16 = np.asarray(inputs["VO"], dtype=np.float32).astype(NPBF16)
    xb16 = [np.ascontiguousarray(x[b].astype(NPBF16)) for b in range(B)]
    in_maps = []
    for c in range(8):
        b, h = divmod(c, 2)
        if h == 0:
            xc = xb16[b]
        else:
            # Rotate rows so this core's queries are rows 0..TQ-1; the key
            # order is permuted identically in S and AV, which softmax and
            # the attention average are invariant to.
            xc = np.concatenate([xb16[b][TQ:], xb16[b][:TQ]], axis=0)
        xTc = np.ascontiguousarray(xc.T)
        in_maps.append({"x": xc, "xT": xTc, "QK": QK16, "VO": VO16})
    return in_maps


def kernel(x, QK, VO):
    in_maps = _make_in_maps({"x": x, "QK": QK, "VO": VO})
    results = _get_runner()(in_maps)
    out = np.empty((B, T, D), dtype=np.float32)
    for c in range(8):
        b, h = divmod(c, 2)
        out[b, h * TQ : (h + 1) * TQ, :] = results[c]["out"].astype(np.float32)
    return out
